# revision 8
# baseline (speedup 1.0000x reference)
"""GCN (Linear+ReLU -> GCNConv+ReLU -> GCNConv -> log_softmax) on 8 Trainium2
NeuronCores via Bass.

Sharding: 1D node partition (6250 nodes/core, padded to 6272). Dense GEMMs run
on each core's node slice with activations kept feature-major ("T layout",
features on partitions). The normalized adjacency is factorized as
D^-1/2 (A+I) D^-1/2, so per-edge weights vanish: each layer scales its
projected features by dinv once (the gather table g = dinv * (h @ W)), the
edge aggregation is a plain unweighted segment sum, and the destination scale
dinv[d] is applied on the way out of PSUM.

Aggregation: the projected/scaled feature table is all-gathered (bf16,
row-padded to 256 B), then each core gathers its in-edges' source rows with
per-edge DMA-gather descriptors. Destinations are packed into 128-node
"windows" sorted by in-degree so the segment sum becomes ELL-style rounds:
each round is one [128 x 128] tile whose partition p belongs to window
position p, accumulated into PSUM with an identity-stationary matmul. The
self-loop term enters PSUM as one extra identity matmul from the local g
tile. Pad slots point at an all-zero table row, so no masking is needed.

Optimizations vs the first working version (1052995 ns -> 607754 ns):
 - window packing sorted by max(dLO,dHI): ~13% fewer ELL rounds; windows
   balanced into uniform call groups (small SBUF gather tiles)
 - both feature tables exchanged in fp8e4m3 PACKED ([*,100] / [*,16]):
   AllGather cost is 15 us + bytes/40GB/s, so 5 MB instead of 12.8 MB;
   packed rows are expanded locally into 256 B-stride gather rows
   (final rel err 3.6e-4, far under the 2e-2 gate)
 - scatter matmuls stream only the real 100/16 columns of the fp8 rows
   (bf16 identity stationary x fp8 moving is legal)
 - STATIC LO/HI node halves (by local id): each layer's table exchange is
   split into two AllGathers; LO-class gathers (which only need the LO
   region) run concurrently with the HI AllGather on the collective cores.
   Scatter phases: LO-class rounds accumulate per-window partials
   (PSUM -> SBUF f32), HI-class rounds + partial combine finish windows.
 - the LO AllGather of layer 1 is emitted mid-phase-A (a few groups after
   its data is ready, so the emission never stalls the x-prefetch queue)
 - layer 2's LO AllGather fires mid-scatter-1 (half-pure call groups make
   all LO-id windows finish early), and scatter-2's LO gathers overlap
   layer 2's HI AllGather
 - phase A fused per column-group (GEMM1+ReLU+GEMM2 in one pass); bulk
   log_softmax in three chunks hidden under scatter-2 gathers, with the five
   lightest windows grouped last so only their tiny chunk is exposed
"""

import sys
from contextlib import ExitStack
from dataclasses import dataclass, field

import numpy as np

sys.path.insert(0, "/opt/trn_rl_repo")

import ml_dtypes  # noqa: E402

BF16 = ml_dtypes.bfloat16

# ---------------------------------------------------------------- config


@dataclass
class Cfg:
    N: int = 50000
    E: int = 800000
    FIN: int = 500
    H1: int = 300
    H2: int = 100
    C: int = 16
    NCORES: int = 8

    FP: int = 512      # padded FIN (contraction tiles of 128)
    H2P: int = 128     # padded H2
    F1C: int = 100     # H1 chunk width (3 chunks of 100)
    CHUNK: int = 1024  # max slots per dma_gather call
    SCRATCH: int = 16384  # SWDGE ring bytes/partition (default; 1024 slots)
    GRPW: int = 4      # windows per call group

    NCR: int = field(init=False)   # real nodes per core
    NCP: int = field(init=False)   # padded nodes per core (x128)
    NW: int = field(init=False)    # windows per core
    NTOT: int = field(init=False)  # padded global table rows
    NLO_W: int = field(init=False)  # windows in the LO half
    LO_N: int = field(init=False)   # LO nodes per core
    LOTOT: int = field(init=False)  # LO table region rows
    HITOT: int = field(init=False)  # HI table region rows

    def __post_init__(self):
        assert self.N % self.NCORES == 0
        self.NCR = self.N // self.NCORES
        self.NCP = ((self.NCR + 127) // 128) * 128
        assert self.NCP > self.NCR, "need at least one pad column per core"
        self.NW = self.NCP // 128
        self.NTOT = self.NCP * self.NCORES
        self.NLO_W = self.NW // 2
        self.LO_N = self.NLO_W * 128
        self.LOTOT = self.LO_N * self.NCORES
        self.HITOT = (self.NCP - self.LO_N) * self.NCORES
        assert max(self.LOTOT, self.HITOT) <= 32767, \
            "table regions must be int16-indexable"
        assert self.H1 % self.F1C == 0


FULL = Cfg()

# ---------------------------------------------------------------- host prep


@dataclass
class Meta:
    """Compile-time structure shared by all cores (SPMD)."""
    calls: list          # (slot_off, n_slots, region 0=LO/1=HI)
    win_lo: list         # per window: [(call_idx, row_in_call), ...] LO class
    win_hi: list         # per window: HI-class chunks
    emit_order: list     # window emission order (group-major)
    SLOTS: int
    NCALL_LO: int        # calls 0..NCALL_LO-1 are the LO phase


def prep_graph(cfg: Cfg, edge_index: np.ndarray):
    """Host-side index preprocessing: sharding, window packing, slot arrays.

    Nodes get a STATIC half assignment (by local id): LO nodes occupy table
    region [0, LOTOT) (= each core's first LO_N positions, concatenated by
    AllGather_a), HI nodes region [LOTOT, NTOT). An edge's gather class is
    its source's half, so all LO-class gathers depend only on AllGather_a —
    they overlap AllGather_b on the collective cores.
    """
    src = edge_index[0].astype(np.int64)
    dst = edge_index[1].astype(np.int64)
    deg = np.bincount(dst, minlength=cfg.N).astype(np.float64) + 1.0
    dinv = (1.0 / np.sqrt(deg)).astype(np.float32)

    NPAD = cfg.NCP - cfg.NCR
    LO_REAL = cfg.LO_N - NPAD // 2       # real LO nodes per core
    score = src // cfg.NCR
    sloc = src % cfg.NCR
    dcore = dst // cfg.NCR
    dloc = dst % cfg.NCR
    lo_src = sloc < LO_REAL              # static class of each edge

    # local id -> half (including pads split between halves)
    own = np.arange(cfg.NCP)
    own_lo = (own < LO_REAL) | ((own >= cfg.NCR) &
                                (own < cfg.NCR + NPAD // 2))
    lo_ids = np.flatnonzero(own_lo)
    hi_ids = np.flatnonzero(~own_lo)
    assert len(lo_ids) == cfg.LO_N

    cores = []
    for c in range(cfg.NCORES):
        m = dcore == c
        cores.append(dict(mask=m, dl=dloc[m], lo=lo_src[m]))

    # pass 1: per-core, per-half window packing sorted by max(dLO,dHI)
    for cc in cores:
        dLO = np.bincount(cc["dl"][cc["lo"]], minlength=cfg.NCP)
        dHI = np.bincount(cc["dl"][~cc["lo"]], minlength=cfg.NCP)
        ordl = lo_ids[np.lexsort((-dHI[lo_ids], -dLO[lo_ids],
                                  -np.maximum(dLO, dHI)[lo_ids]))]
        ordh = hi_ids[np.lexsort((-dHI[hi_ids], -dLO[hi_ids],
                                  -np.maximum(dLO, dHI)[hi_ids]))]
        cc.update(dLO=dLO, dHI=dHI, order0=np.concatenate([ordl, ordh]))

    # shared per-bin round maxima (bin w = order0[w*128:(w+1)*128])
    RLO0 = np.zeros(cfg.NW, np.int64)
    RHI0 = np.zeros(cfg.NW, np.int64)
    for cc in cores:
        RLO0 = np.maximum(RLO0, cc["dLO"][cc["order0"]].reshape(cfg.NW, 128).max(1))
        RHI0 = np.maximum(RHI0, cc["dHI"][cc["order0"]].reshape(cfg.NW, 128).max(1))
    RLO0 = np.maximum(RLO0, 1)

    # half-pure call groups (LO-id windows never share a group with HI-id
    # windows), balanced by load within each half. In the HI phase the
    # LO-id groups' spans are gathered FIRST so all LO-id windows finish
    # early: that releases the layer-2 LO AllGather mid-scatter.
    def balance(bins, loads_):
        ng = (len(bins) + cfg.GRPW - 1) // cfg.GRPW
        gl = np.zeros(ng, np.int64)
        gs = [[] for _ in range(ng)]
        for b in sorted(bins, key=lambda b: -loads_[b]):
            gi = min((g for g in range(ng) if len(gs[g]) < cfg.GRPW),
                     key=lambda g: gl[g])
            gs[gi].append(int(b))
            gl[gi] += loads_[b]
        return gs

    loads = RLO0 + RHI0
    groups_lo = balance(range(cfg.NLO_W), loads)
    # the 5 lightest HI bins form their own FINAL group so the last
    # emission (and its softmax chunk) is as small as possible
    groups_hi = balance(range(cfg.NLO_W, cfg.NW - 5), loads)
    groups_hi.append(list(range(cfg.NW - 5, cfg.NW)))
    groups = groups_lo + groups_hi
    RLO, RHI = RLO0, RHI0

    # final per-core node order is just order0 (bin w = window w)
    for cc in cores:
        order = cc["order0"]
        qpos = np.empty(cfg.NCP, np.int64)
        qpos[order] = np.arange(cfg.NCP)
        cc.update(order=order, qpos=qpos)

    # global slot layout: LO phase (per group, LO spans), then HI phase
    calls = []
    win_lo = [[] for _ in range(cfg.NW)]
    win_hi = [[] for _ in range(cfg.NW)]
    off = 0
    NCALL_LO = 0
    for region, RR, win_c in ((0, RLO, win_lo), (1, RHI, win_hi)):
        for g in groups:
            span_rows = [(wdw, r) for wdw in g for r in range(int(RR[wdw]))]
            rows = len(span_rows)
            if not rows:
                continue
            ncall = (rows * 128 + cfg.CHUNK - 1) // cfg.CHUNK
            per = (rows + ncall - 1) // ncall
            r0 = 0
            while r0 < rows:
                n_rows = min(per, rows - r0)
                ci = len(calls)
                calls.append((off, n_rows * 128, region))
                for rr in range(n_rows):
                    wdw, _ = span_rows[r0 + rr]
                    win_c[wdw].append((ci, rr))
                off += n_rows * 128
                r0 += n_rows
        if region == 0:
            NCALL_LO = len(calls)
    SLOTS = off
    assert SLOTS % 16 == 0

    emit_order = [w for g in groups for w in g]
    meta = Meta(calls=calls, win_lo=win_lo, win_hi=win_hi,
                emit_order=emit_order, SLOTS=SLOTS, NCALL_LO=NCALL_LO)

    # pass 2: fill per-core slot index arrays.
    # region-local table row of source (c, local q0):
    #   LO: c*LO_N + qpos[q0]            (qpos < LO_N)
    #   HI: c*(NCP-LO_N) + qpos[q0]-LO_N
    qpos_all = np.concatenate([cc["qpos"] for cc in cores])
    qp_src = qpos_all[score * cfg.NCP + sloc]
    HI_N = cfg.NCP - cfg.LO_N
    grow = np.where(lo_src, score * cfg.LO_N + qp_src,
                    score * HI_N + qp_src - cfg.LO_N)

    zrowLO = int(cores[0]["qpos"][cfg.NCR])            # core0 LO pad
    zrowHI = int(cores[0]["qpos"][cfg.NCR + NPAD // 2]) - cfg.LO_N
    assert 0 <= zrowLO < cfg.LO_N and 0 <= zrowHI < HI_N

    # global slot of each (window, class round)
    rsL = np.zeros((cfg.NW, int(RLO.max())), np.int64)
    rsH = np.zeros((cfg.NW, max(int(RHI.max()), 1)), np.int64)
    for wdw in range(cfg.NW):
        for r, (ci, rr) in enumerate(win_lo[wdw]):
            rsL[wdw, r] = calls[ci][0] + rr * 128
        for r, (ci, rr) in enumerate(win_hi[wdw]):
            rsH[wdw, r] = calls[ci][0] + rr * 128

    for c in range(cfg.NCORES):
        cc = cores[c]
        m = cc["mask"]
        eg = grow[m]
        elo = cc["lo"]
        edl = cc["dl"]
        ew = cc["qpos"][edl] // 128
        ep = cc["qpos"][edl] % 128
        # rank of edge within its (dloc, class) group
        key = edl * 2 + (~elo).astype(np.int64)
        o = np.argsort(key, kind="stable")
        ks = key[o]
        first = np.r_[0, np.flatnonzero(ks[1:] != ks[:-1]) + 1]
        starts = np.zeros(len(ks), np.int64)
        starts[first] = first
        starts = np.maximum.accumulate(starts)
        rank = np.empty(len(ks), np.int64)
        rank[o] = np.arange(len(ks)) - starts

        idxv = np.full(SLOTS, -1, np.int64)
        for coff, n, region in calls:
            idxv[coff:coff + n] = zrowLO if region == 0 else zrowHI
        posL = rsL[ew, np.minimum(rank, rsL.shape[1] - 1)] + ep
        posH = rsH[ew, np.minimum(rank, rsH.shape[1] - 1)] + ep
        pos = np.where(elo, posL, posH)
        idxv[pos] = eg
        assert idxv.min() >= 0
        assert idxv[np.concatenate([np.arange(co, co + n)
                    for co, n, rg in calls if rg == 0])].max() < cfg.LOTOT
        gi = idxv.reshape(-1, 16).T.astype(np.int16)         # [16, SLOTS/16]
        cc["gidx"] = np.tile(gi, (8, 1))                     # [128, SLOTS/16]
    return dinv, cores, meta


def prep_inputs(cfg: Cfg, inputs: dict, dinv, cores, meta: Meta):
    """Build per-core in_maps (numpy) for the device kernel."""
    x = np.asarray(inputs["x"], np.float32)
    lin_W = np.asarray(inputs["lin_W"], np.float32)
    lin_b = np.asarray(inputs["lin_b"], np.float32)
    W1 = np.asarray(inputs["W1"], np.float32)
    b1 = np.asarray(inputs["b1"], np.float32)
    W2 = np.asarray(inputs["W2"], np.float32)
    b2 = np.asarray(inputs["b2"], np.float32)

    linWp = np.zeros((cfg.FP, cfg.H1), BF16)
    linWp[:cfg.FIN] = lin_W.astype(BF16)
    nf1 = cfg.H1 // cfg.F1C
    linbp = lin_b.reshape(nf1, cfg.F1C).T.astype(np.float32).copy()  # [F1C, nf1]
    W1p = np.zeros((cfg.H1, cfg.H2P), BF16)
    W1p[:, :cfg.H2] = W1.astype(BF16)
    W2p = np.zeros((cfg.H2P, cfg.C), BF16)
    W2p[:cfg.H2] = W2.astype(BF16)
    b1rep = np.zeros((128, cfg.H2P), np.float32)
    b1rep[:, :cfg.H2] = b1
    b2rep = np.tile(b2.reshape(1, cfg.C), (128, 1)).astype(np.float32)
    ident = np.eye(128, dtype=BF16)

    xT = np.zeros((cfg.FP, cfg.N), np.float32)
    xT[:cfg.FIN] = x.T

    in_maps = []
    for c in range(cfg.NCORES):
        cc = cores[c]
        order = cc["order"]
        real = order < cfg.NCR
        gcols = np.where(real, cfg.NCR * c + np.minimum(order, cfg.NCR - 1), 0)
        xTc = xT[:, gcols] * real[None, :]
        dv = dinv[gcols] * real
        dinvT = np.tile(dv.astype(BF16).reshape(1, -1), (128, 1))
        dinvN = dv.reshape(cfg.NW, 128).T.astype(np.float32).copy()
        in_maps.append({
            "xT": xTc.astype(BF16),
            "linW": linWp, "linb": linbp,
            "W1": W1p, "W2": W2p,
            "b1rep": b1rep, "b2rep": b2rep,
            "dinvT": dinvT, "dinvN": dinvN,
            "ident": ident, "gidx": cc["gidx"],
        })
    return in_maps


def assemble_output(cfg: Cfg, cores, outs):
    """outs: per-core [128, NW*C] -> full [N, C] float32."""
    res = np.empty((cfg.N, cfg.C), np.float32)
    for c in range(cfg.NCORES):
        o = np.asarray(outs[c]).reshape(128, cfg.NW, cfg.C)
        o = o.transpose(1, 0, 2).reshape(cfg.NCP, cfg.C)  # device node order
        order = cores[c]["order"]
        real = order < cfg.NCR
        res[c * cfg.NCR + order[real]] = o[real]
    return res


# ---------------------------------------------------------------- device kernel


def build_nc(cfg: Cfg, meta: Meta):
    import concourse.bacc as bacc
    import concourse.mybir as mybir
    import concourse.tile as tile

    dt = mybir.dt
    f32, bf16, i16 = dt.float32, dt.bfloat16, dt.int16
    fp8 = dt.float8e4
    AF = mybir.ActivationFunctionType
    OP = mybir.AluOpType

    nc = bacc.Bacc("TRN2", target_bir_lowering=False, debug=False,
                   enable_asserts=False, num_devices=cfg.NCORES,
                   num_swdge_queues=2,
                   dynamic_dma_scratch_size=cfg.SCRATCH)

    NCP, NW, NTOT, C = cfg.NCP, cfg.NW, cfg.NTOT, cfg.C
    LO_N, LOTOT, HITOT = cfg.LO_N, cfg.LOTOT, cfg.HITOT
    NLO_W = cfg.NLO_W
    F1C, H2P, H2 = cfg.F1C, cfg.H2P, cfg.H2
    NK = cfg.FP // 128          # contraction tiles for GEMM1
    NF1 = cfg.H1 // F1C         # feature chunks of h1

    xT_d = nc.dram_tensor("xT", [cfg.FP, NCP], bf16, kind="ExternalInput")
    linW_d = nc.dram_tensor("linW", [cfg.FP, cfg.H1], bf16, kind="ExternalInput")
    linb_d = nc.dram_tensor("linb", [F1C, NF1], f32, kind="ExternalInput")
    W1_d = nc.dram_tensor("W1", [cfg.H1, H2P], bf16, kind="ExternalInput")
    W2_d = nc.dram_tensor("W2", [H2P, C], bf16, kind="ExternalInput")
    b1r_d = nc.dram_tensor("b1rep", [128, H2P], f32, kind="ExternalInput")
    b2r_d = nc.dram_tensor("b2rep", [128, C], f32, kind="ExternalInput")
    dvT_d = nc.dram_tensor("dinvT", [128, NCP], bf16, kind="ExternalInput")
    dvN_d = nc.dram_tensor("dinvN", [128, NW], f32, kind="ExternalInput")
    id_d = nc.dram_tensor("ident", [128, 128], bf16, kind="ExternalInput")
    gi_d = nc.dram_tensor("gidx", [128, meta.SLOTS // 16], i16, kind="ExternalInput")
    out_d = nc.dram_tensor("out", [128, NW * C], f32, kind="ExternalOutput")

    GW = [(i, min(512, NCP - i)) for i in range(0, NCP, 512)]
    CROWS = cfg.CHUNK // 128
    outv = out_d[:].rearrange("p (w c) -> p w c", c=C)

    with tile.TileContext(nc) as tc, ExitStack() as top:
        const = top.enter_context(tc.tile_pool(name="const", bufs=1))
        dram = top.enter_context(tc.tile_pool(name="dram", bufs=1, space="DRAM"))

        ident = const.tile([128, 128], bf16)
        nc.sync.dma_start(ident[:], id_d[:])
        dinvT = const.tile([128, NCP], bf16)
        nc.sync.dma_start(dinvT[:], dvT_d[:])
        dinvN = const.tile([128, NW], f32)
        nc.sync.dma_start(dinvN[:], dvN_d[:])
        b1rep = const.tile([128, H2P], f32)
        nc.sync.dma_start(b1rep[:], b1r_d[:])
        b2rep = const.tile([128, C], f32)
        nc.sync.dma_start(b2rep[:], b2r_d[:])
        linb = const.tile([F1C, NF1], f32)
        nc.sync.dma_start(linb[:], linb_d[:])
        gidx = const.tile([128, meta.SLOTS // 16], i16)
        nc.sync.dma_start(gidx[:], gi_d[:])
        W1t = []
        for f in range(NF1):
            t = const.tile([F1C, H2P], bf16, name=f"W1t{f}")
            nc.sync.dma_start(t[:], W1_d[f * F1C:(f + 1) * F1C, :])
            W1t.append(t)
        W2t = const.tile([H2P, C], bf16)
        nc.sync.dma_start(W2t[:], W2_d[:])

        g1T = const.tile([128, NCP], bf16, tag="bigA")
        g1nat = const.tile([128, NW, 128], bf16, tag="bigB")
        g1nat8 = const.tile([128, NW, H2], fp8)
        h2nat = const.tile([128, NW, H2P], bf16)
        h2T = const.tile([128, NW, 128], bf16, tag="bigA")  # [f, w, p]
        g2Tf = const.tile([128, NCP], bf16)
        g2nat = const.tile([128, NW, 128], bf16, tag="bigB")
        g2nat8 = const.tile([128, NW, C], fp8)
        logit = const.tile([128, NW, C], f32)
        outsb = const.tile([128, NW, C], f32)
        h1part = const.tile([128, NW, H2], f32)
        l2part = const.tile([128, NW, C], f32)

        g1loc8a = dram.tile([LO_N, H2], fp8)
        g1loc8b = dram.tile([NCP - LO_N, H2], fp8)
        g2loc8a = dram.tile([LO_N, C], fp8)
        g2loc8b = dram.tile([NCP - LO_N, C], fp8)
        full1p8a = dram.tile([LOTOT, H2], fp8, addr_space="Shared")
        full1p8b = dram.tile([HITOT, H2], fp8, addr_space="Shared")
        full18 = dram.tile([NTOT, 256], fp8)
        full2p8a = dram.tile([LOTOT, C], fp8, addr_space="Shared")
        full2p8b = dram.tile([HITOT, C], fp8, addr_space="Shared")
        full28 = dram.tile([NTOT, 256], fp8)

        # garbage-free upper feature columns for the h2 transpose / GEMM3
        nc.gpsimd.memset(h2nat[:], 0.0)
        nc.gpsimd.memset(g2Tf[:], 0.0)

        # ---- phase A: fused GEMM1 (relu(x@linW+b)) + GEMM2 (g1 = dinv*(h1@W1)),
        # with the g1 transpose/fp8-convert/table-write interleaved per group
        with ExitStack() as ph:
            xp = ph.enter_context(tc.tile_pool(name="xp", bufs=3))
            hp = ph.enter_context(tc.tile_pool(name="hp", bufs=2))
            psA = ph.enter_context(tc.tile_pool(name="psA", bufs=2, space="PSUM"))
            psB = ph.enter_context(tc.tile_pool(name="psB", bufs=2, space="PSUM"))
            lw = ph.enter_context(tc.tile_pool(name="lw", bufs=1))
            lwt = []
            for k in range(NK):
                t = lw.tile([128, cfg.H1], bf16, name=f"lwt{k}")
                nc.sync.dma_start(t[:], linW_d[k * 128:(k + 1) * 128, :])
                lwt.append(t)
            xTv = xT_d[:].rearrange("(k p) c -> p k c", p=128)
            for (c0, cw) in GW:
                xg = xp.tile([128, NK, 512], bf16, tag="xg")
                nc.sync.dma_start(xg[:, :, :cw], xTv[:, :, c0:c0 + cw])
                h1g = hp.tile([F1C, NF1, 512], bf16, tag="h1g")
                for f in range(NF1):
                    acc = psA.tile([F1C, 512], f32, tag="accA")
                    for k in range(NK):
                        nc.tensor.matmul(
                            acc[:, :cw],
                            lwt[k][:, f * F1C:(f + 1) * F1C],
                            xg[:, k, :cw],
                            start=(k == 0), stop=(k == NK - 1))
                    nc.scalar.activation(h1g[:, f, :cw], acc[:, :cw],
                                         AF.Relu, bias=linb[:, f:f + 1])
                accB = psB.tile([H2P, 512], f32, tag="accB")
                for f in range(NF1):
                    nc.tensor.matmul(accB[:, :cw], W1t[f][:],
                                     h1g[:, f, :cw],
                                     start=(f == 0), stop=(f == NF1 - 1))
                nc.vector.tensor_mul(g1T[:, c0:c0 + cw], accB[:, :cw],
                                     dinvT[:, c0:c0 + cw])
                if c0 == LO_N + 1536:
                    # LO half of the table completed a few groups ago; its
                    # tail ops' waits are satisfied by now, so emitting them
                    # here does not stall the x prefetch queue
                    nc.sync.dma_start_transpose(g1nat[:, :NLO_W, :],
                                                g1T[:, :LO_N])
                    nc.vector.tensor_copy(g1nat8[:, :NLO_W, :],
                                          g1nat[:, :NLO_W, :H2])
                    nc.sync.dma_start(
                        g1loc8a[:].rearrange("(w p) f -> p w f", p=128),
                        g1nat8[:, :NLO_W, :])
                    nc.gpsimd.collective_compute(
                        "AllGather", OP.bypass,
                        replica_groups=[list(range(cfg.NCORES))],
                        ins=[g1loc8a[:]], outs=[full1p8a[:]])

        # b-chain on the Activation HWDGE queue: the SP queue's scheduler
        # barriers would otherwise serialize it behind AllGather_a
        nc.scalar.dma_start_transpose(g1nat[:, NLO_W:, :], g1T[:, LO_N:])
        nc.vector.tensor_copy(g1nat8[:, NLO_W:, :], g1nat[:, NLO_W:, :H2])
        nc.scalar.dma_start(
            g1loc8b[:].rearrange("(w p) f -> p w f", p=128),
            g1nat8[:, NLO_W:, :])
        nc.gpsimd.collective_compute(
            "AllGather", OP.bypass,
            replica_groups=[list(range(cfg.NCORES))],
            ins=[g1loc8b[:]], outs=[full1p8b[:]])
        # expand packed rows into 256 B-stride gather rows; LO expand runs
        # during AllGather_b, so LO-class gathers also overlap AllGather_b
        nc.sync.dma_start(full18[:LOTOT, :H2], full1p8a[:])
        nc.sync.dma_start(full18[LOTOT:, :H2], full1p8b[:])

        # ---- log_softmax over C for a window range (bulk: 3 act-table
        # loads per chunk, not per window)
        def softmax_chunk(sp, w0, w1):
            nwc = w1 - w0
            et = sp.tile([128, NW, C], f32, tag="et")
            nc.scalar.activation(et[:, :nwc, :], logit[:, w0:w1, :], AF.Exp)
            ssum = sp.tile([128, NW], f32, tag="ssum")
            nc.vector.tensor_reduce(ssum[:, :nwc], et[:, :nwc, :],
                                    mybir.AxisListType.X, OP.add)
            negl = sp.tile([128, NW], f32, tag="negl")
            nc.scalar.activation(negl[:, :nwc], ssum[:, :nwc], AF.Ln)
            nc.vector.tensor_scalar_mul(negl[:, :nwc], negl[:, :nwc], -1.0)
            for w in range(w0, w1):
                nc.scalar.activation(outsb[:, w, :], logit[:, w, :],
                                     AF.Identity, bias=negl[:, w - w0:w - w0 + 1])
            nc.sync.dma_start(outv[:, w0:w1, :], outsb[:, w0:w1, :])

        # layer-2 projection chain for a window range: h2 -> h2T -> GEMM3 ->
        # g2 (T) -> g2 natural -> fp8 -> local table slice
        ps3 = top.enter_context(tc.tile_pool(name="ps3", bufs=2, space="PSUM"))

        def g2_chain(w0, w1):
            nc.sync.dma_start_transpose(
                h2T[:, w0:w1, :],
                h2nat[:, w0:w1, :].rearrange("p w f -> p (w f)"))
            h2Tf = h2T[:].rearrange("f w p -> f (w p)")
            for c0 in range(w0 * 128, w1 * 128, 512):
                cw = min(512, w1 * 128 - c0)
                acc = ps3.tile([C, 512], f32, tag="acc3")
                nc.tensor.matmul(acc[:, :cw], W2t[:], h2Tf[:, c0:c0 + cw],
                                 start=True, stop=True)
                nc.vector.tensor_mul(g2Tf[:C, c0:c0 + cw], acc[:, :cw],
                                     dinvT[:C, c0:c0 + cw])
            nc.sync.dma_start_transpose(g2nat[:, w0:w1, :],
                                        g2Tf[:, w0 * 128:w1 * 128])
            nc.vector.tensor_copy(g2nat8[:, w0:w1, :],
                                  g2nat[:, w0:w1, :C])
            g2dst = g2loc8a if w1 <= NLO_W else g2loc8b
            woff = 0 if w1 <= NLO_W else NLO_W
            nc.sync.dma_start(
                g2dst[(w0 - woff) * 128:(w1 - woff) * 128, :]
                .rearrange("(w p) c -> p w c", p=128),
                g2nat8[:, w0:w1, :])

        def emit_ag2a():
            nc.gpsimd.collective_compute(
                "AllGather", OP.bypass,
                replica_groups=[list(range(cfg.NCORES))],
                ins=[g2loc8a[:]], outs=[full2p8a[:]])

        # ---- scatter layers (two phases: LO-class gathers into per-window
        # partials, then HI-class gathers + combine)
        def scatter(full, gnat, layer, fw, part):
            with ExitStack() as ph:
                gp = ph.enter_context(tc.tile_pool(name=f"gb{layer}", bufs=5))
                pp = ph.enter_context(tc.tile_pool(name=f"psW{layer}", bufs=4,
                                                   space="PSUM"))
                ep = ph.enter_context(tc.tile_pool(name=f"ep{layer}", bufs=6))
                sp = ph.enter_context(tc.tile_pool(name=f"sm{layer}", bufs=1))
                wcut = NW - 5               # last softmax chunk (light group)
                gtiles = {}
                lo_done = hi_done = cnt_loid = 0
                cnt_a = cnt_b = 0
                defer_ag2a = -1
                for ci, (coff, n, region) in enumerate(meta.calls):
                    t = gp.tile([128, CROWS, 256], fp8, tag="gb")
                    src = full[:LOTOT, :] if region == 0 else full[LOTOT:, :]
                    nc.gpsimd.dma_gather(
                        t[:, :n // 128, :], src,
                        gidx[:, coff // 16:(coff + n) // 16],
                        num_idxs=n, num_idxs_reg=n, elem_size=256,
                        queue_num=ci % 2)
                    gtiles[ci] = t
                    if defer_ag2a > 0:
                        defer_ag2a -= 1
                        if defer_ag2a == 0:
                            emit_ag2a()
                    if region == 0:
                        # accumulate self + LO rounds into the partial
                        while lo_done < NW and all(
                                c <= ci for c, _ in
                                meta.win_lo[meta.emit_order[lo_done]]):
                            w = meta.emit_order[lo_done]
                            chunks = meta.win_lo[w]
                            acc = pp.tile([128, fw], f32, tag="pw")
                            nc.tensor.matmul(acc[:], ident[:], gnat[:, w, :fw],
                                             start=True, stop=not chunks)
                            for k, (cidx, row) in enumerate(chunks):
                                nc.tensor.matmul(
                                    acc[:], ident[:], gtiles[cidx][:, row, :fw],
                                    start=False, stop=(k == len(chunks) - 1))
                            nc.vector.tensor_copy(part[:, w, :], acc[:])
                            lo_done += 1
                        continue
                    while hi_done < NW and all(
                            c <= ci for c, _ in
                            meta.win_hi[meta.emit_order[hi_done]]):
                        w = meta.emit_order[hi_done]
                        chunks = meta.win_hi[w]
                        if chunks:
                            acc = pp.tile([128, fw], f32, tag="pw")
                            for k, (cidx, row) in enumerate(chunks):
                                nc.tensor.matmul(
                                    acc[:], ident[:], gtiles[cidx][:, row, :fw],
                                    start=(k == 0), stop=(k == len(chunks) - 1))
                            s = ep.tile([128, fw], f32, tag="s")
                            nc.vector.scalar_tensor_tensor(
                                s[:], acc[:], 0.0, part[:, w, :],
                                OP.add, OP.add)
                            sv = s[:]
                        else:
                            sv = part[:, w, :]
                        if layer == 1:
                            t2 = ep.tile([128, H2], f32, tag="t2")
                            nc.vector.scalar_tensor_tensor(
                                t2[:], sv, dinvN[:, w:w + 1],
                                b1rep[:, :H2], OP.mult, OP.add)
                            nc.scalar.activation(h2nat[:, w, :H2], t2[:],
                                                 AF.Relu)
                            if w < NLO_W:
                                cnt_loid += 1
                                if cnt_loid == NLO_W:
                                    # all LO-id windows done: project their
                                    # g2 slice; the AllGather is emitted a
                                    # few calls later so its input wait does
                                    # not stall the Pool gather queue
                                    g2_chain(0, NLO_W)
                                    defer_ag2a = 6
                        else:
                            nc.vector.scalar_tensor_tensor(
                                logit[:, w, :], sv, dinvN[:, w:w + 1],
                                b2rep[:], OP.mult, OP.add)
                            if w < NLO_W:
                                cnt_a += 1
                                if cnt_a == NLO_W:
                                    softmax_chunk(sp, 0, NLO_W)
                            elif w < wcut:
                                cnt_b += 1
                                if cnt_b == wcut - NLO_W:
                                    softmax_chunk(sp, NLO_W, wcut)
                        hi_done += 1
                assert lo_done == NW and hi_done == NW
                if layer == 1 and defer_ag2a > 0:
                    emit_ag2a()
                if layer == 2:
                    softmax_chunk(sp, wcut, NW)

        scatter(full18, g1nat, layer=1, fw=H2, part=h1part)

        # remaining HI-id window projection + its AllGather; the LO expand
        # (and then scatter-2's LO gathers) overlap AllGather_b of layer 2
        g2_chain(NLO_W, NW)
        nc.gpsimd.collective_compute(
            "AllGather", OP.bypass,
            replica_groups=[list(range(cfg.NCORES))],
            ins=[g2loc8b[:]], outs=[full2p8b[:]])
        nc.sync.dma_start(full28[:LOTOT, :C], full2p8a[:])
        nc.sync.dma_start(full28[LOTOT:, :C], full2p8b[:])

        scatter(full28, g2nat, layer=2, fw=C, part=l2part)

    nc.compile()
    return nc


# ---------------------------------------------------------------- entry

_CACHE = {}


def _get_nc(cfg: Cfg, meta: Meta):
    key = (cfg.N, cfg.E, meta.SLOTS, tuple(tuple(c) for c in meta.calls))
    if key not in _CACHE:
        _CACHE[key] = build_nc(cfg, meta)
    return _CACHE[key]


def run(cfg: Cfg, inputs: dict, trace: bool = False):
    from concourse.bass_utils import run_bass_kernel_spmd
    dinv, cores, meta = prep_graph(cfg, np.asarray(inputs["edge_index"]))
    in_maps = prep_inputs(cfg, inputs, dinv, cores, meta)
    nc = _get_nc(cfg, meta)
    try:
        res = run_bass_kernel_spmd(nc, in_maps,
                                   core_ids=list(range(cfg.NCORES)),
                                   trace=trace)
    except ModuleNotFoundError:
        res = run_bass_kernel_spmd(nc, in_maps,
                                   core_ids=list(range(cfg.NCORES)),
                                   trace=False)
    out = assemble_output(cfg, cores, [r["out"] for r in res.results])
    return out, res


def kernel(**inputs) -> np.ndarray:
    out, _ = run(FULL, inputs)
    return out


def bench_chain(cfg: Cfg, inputs: dict, iters: int = 8):
    """Time device execution by chaining `iters` NEFF executions in one jit
    (output of run k feeds the donated output buffer of run k+1, serializing
    them); returns (per_exec_seconds, outputs_of_last_run)."""
    import time as _time

    import jax
    import numpy as _np
    from jax.experimental.shard_map import shard_map
    from jax.sharding import Mesh, PartitionSpec

    import concourse.mybir as mybir
    from concourse import bass2jax

    dinv, cores, meta = prep_graph(cfg, np.asarray(inputs["edge_index"]))
    in_maps = prep_inputs(cfg, inputs, dinv, cores, meta)
    nc = _get_nc(cfg, meta)
    bass2jax.install_neuronx_cc_hook()

    pname = nc.partition_id_tensor.name if nc.partition_id_tensor else None
    in_names, out_names, out_avals, zero_outs = [], [], [], []
    for alloc in nc.m.functions[0].allocations:
        if not isinstance(alloc, mybir.MemoryLocationSet):
            continue
        name = alloc.memorylocations[0].name
        if alloc.kind == "ExternalInput":
            if name != pname:
                in_names.append(name)
        elif alloc.kind == "ExternalOutput":
            out_names.append(name)
            shape = tuple(alloc.tensor_shape)
            dtype = mybir.dt.np(alloc.dtype)
            out_avals.append(jax.core.ShapedArray(shape, dtype))
            zero_outs.append(_np.zeros(shape, dtype))
    n_params = len(in_names)
    all_names = in_names + out_names + ([pname] if pname else [])

    def _body_n(n_execs, *args):
        operands = list(args[:n_params])
        outs = list(args[n_params:])
        pid = [bass2jax.partition_id_tensor()] if pname else []
        for _ in range(n_execs):
            outs = list(bass2jax._bass_exec_p.bind(
                *operands, *outs, *pid,
                out_avals=tuple(out_avals),
                in_names=tuple(all_names),
                out_names=tuple(out_names),
                lowering_input_output_aliases=(),
                sim_require_finite=True, sim_require_nnan=True, nc=nc))
        return tuple(outs)

    devices = jax.devices()[:cfg.NCORES]
    mesh = Mesh(_np.asarray(devices), ("core",))
    spec = (PartitionSpec("core"),)
    concat_in = [_np.concatenate([_np.asarray(in_maps[c][n])
                                  for c in range(cfg.NCORES)], axis=0)
                 for n in in_names]
    concat_zeros = [_np.zeros((cfg.NCORES * z.shape[0], *z.shape[1:]), z.dtype)
                    for z in zero_outs]
    nin = n_params + len(zero_outs)
    fn = jax.jit(
        shard_map(lambda *a: _body_n(1, *a), mesh=mesh,
                  in_specs=spec * nin, out_specs=spec * len(out_names),
                  check_rep=False),
        donate_argnums=tuple(range(n_params, nin)), keep_unused=True)
    din = [jax.device_put(x) for x in concat_in]
    outs = fn(*din, *concat_zeros)  # compile+warm
    jax.block_until_ready(outs)
    zzs = []
    for _ in range(iters):
        zzs.append([jax.device_put(
            _np.zeros((cfg.NCORES * z.shape[0], *z.shape[1:]), z.dtype))
            for z in zero_outs])
    jax.block_until_ready(zzs)
    t0 = _time.perf_counter()
    all_outs = [fn(*din, *zz) for zz in zzs]
    jax.block_until_ready(all_outs)
    t_pipe = _time.perf_counter() - t0
    times = [t_pipe / iters]
    per_exec = times[0]
    last = all_outs[-1]
    outs_np = [_np.asarray(last[i]).reshape(cfg.NCORES, *out_avals[i].shape)
               for i in range(len(out_names))]
    out = assemble_output(cfg, cores,
                          [outs_np[out_names.index("out")][c]
                           for c in range(cfg.NCORES)])
    return per_exec, times, out



# revision 9
# speedup vs baseline: 1.0043x; 1.0043x over previous
"""GCN (Linear+ReLU -> GCNConv+ReLU -> GCNConv -> log_softmax) on 8 Trainium2
NeuronCores via Bass.

Sharding: 1D node partition (6250 nodes/core, padded to 6272). Dense GEMMs run
on each core's node slice with activations kept feature-major ("T layout",
features on partitions). The normalized adjacency is factorized as
D^-1/2 (A+I) D^-1/2, so per-edge weights vanish: each layer scales its
projected features by dinv once (the gather table g = dinv * (h @ W)), the
edge aggregation is a plain unweighted segment sum, and the destination scale
dinv[d] is applied on the way out of PSUM.

Aggregation: the projected/scaled feature table is all-gathered (bf16,
row-padded to 256 B), then each core gathers its in-edges' source rows with
per-edge DMA-gather descriptors. Destinations are packed into 128-node
"windows" sorted by in-degree so the segment sum becomes ELL-style rounds:
each round is one [128 x 128] tile whose partition p belongs to window
position p, accumulated into PSUM with an identity-stationary matmul. The
self-loop term enters PSUM as one extra identity matmul from the local g
tile. Pad slots point at an all-zero table row, so no masking is needed.

Optimizations vs the first working version (1052995 ns -> 607754 ns):
 - window packing sorted by max(dLO,dHI): ~13% fewer ELL rounds; windows
   balanced into uniform call groups (small SBUF gather tiles)
 - both feature tables exchanged in fp8e4m3 PACKED ([*,100] / [*,16]):
   AllGather cost is 15 us + bytes/40GB/s, so 5 MB instead of 12.8 MB;
   packed rows are expanded locally into 256 B-stride gather rows
   (final rel err 3.6e-4, far under the 2e-2 gate)
 - scatter matmuls stream only the real 100/16 columns of the fp8 rows
   (bf16 identity stationary x fp8 moving is legal)
 - STATIC LO/HI node halves (by local id): each layer's table exchange is
   split into two AllGathers; LO-class gathers (which only need the LO
   region) run concurrently with the HI AllGather on the collective cores.
   Scatter phases: LO-class rounds accumulate per-window partials
   (PSUM -> SBUF f32), HI-class rounds + partial combine finish windows.
 - the LO AllGather of layer 1 is emitted mid-phase-A (a few groups after
   its data is ready, so the emission never stalls the x-prefetch queue)
 - layer 2's LO AllGather fires mid-scatter-1 (half-pure call groups make
   all LO-id windows finish early), and scatter-2's LO gathers overlap
   layer 2's HI AllGather
 - phase A fused per column-group (GEMM1+ReLU+GEMM2 in one pass); bulk
   log_softmax in three chunks hidden under scatter-2 gathers, with the five
   lightest windows grouped last so only their tiny chunk is exposed
"""

import sys
from contextlib import ExitStack
from dataclasses import dataclass, field

import numpy as np

sys.path.insert(0, "/opt/trn_rl_repo")

import ml_dtypes  # noqa: E402

BF16 = ml_dtypes.bfloat16

# ---------------------------------------------------------------- config


@dataclass
class Cfg:
    N: int = 50000
    E: int = 800000
    FIN: int = 500
    H1: int = 300
    H2: int = 100
    C: int = 16
    NCORES: int = 8

    FP: int = 512      # padded FIN (contraction tiles of 128)
    H2P: int = 128     # padded H2
    F1C: int = 100     # H1 chunk width (3 chunks of 100)
    CHUNK: int = 1024  # max slots per dma_gather call
    SCRATCH: int = 16384  # SWDGE ring bytes/partition (default; 1024 slots)
    GRPW: int = 6      # windows per call group

    NCR: int = field(init=False)   # real nodes per core
    NCP: int = field(init=False)   # padded nodes per core (x128)
    NW: int = field(init=False)    # windows per core
    NTOT: int = field(init=False)  # padded global table rows
    NLO_W: int = field(init=False)  # windows in the LO half
    LO_N: int = field(init=False)   # LO nodes per core
    LOTOT: int = field(init=False)  # LO table region rows
    HITOT: int = field(init=False)  # HI table region rows

    def __post_init__(self):
        assert self.N % self.NCORES == 0
        self.NCR = self.N // self.NCORES
        self.NCP = ((self.NCR + 127) // 128) * 128
        assert self.NCP > self.NCR, "need at least one pad column per core"
        self.NW = self.NCP // 128
        self.NTOT = self.NCP * self.NCORES
        self.NLO_W = self.NW // 2
        self.LO_N = self.NLO_W * 128
        self.LOTOT = self.LO_N * self.NCORES
        self.HITOT = (self.NCP - self.LO_N) * self.NCORES
        assert max(self.LOTOT, self.HITOT) <= 32767, \
            "table regions must be int16-indexable"
        assert self.H1 % self.F1C == 0


FULL = Cfg()

# ---------------------------------------------------------------- host prep


@dataclass
class Meta:
    """Compile-time structure shared by all cores (SPMD)."""
    calls: list          # (slot_off, n_slots, region 0=LO/1=HI)
    win_lo: list         # per window: [(call_idx, row_in_call), ...] LO class
    win_hi: list         # per window: HI-class chunks
    emit_order: list     # window emission order (group-major)
    SLOTS: int
    NCALL_LO: int        # calls 0..NCALL_LO-1 are the LO phase


def prep_graph(cfg: Cfg, edge_index: np.ndarray):
    """Host-side index preprocessing: sharding, window packing, slot arrays.

    Nodes get a STATIC half assignment (by local id): LO nodes occupy table
    region [0, LOTOT) (= each core's first LO_N positions, concatenated by
    AllGather_a), HI nodes region [LOTOT, NTOT). An edge's gather class is
    its source's half, so all LO-class gathers depend only on AllGather_a —
    they overlap AllGather_b on the collective cores.
    """
    src = edge_index[0].astype(np.int64)
    dst = edge_index[1].astype(np.int64)
    deg = np.bincount(dst, minlength=cfg.N).astype(np.float64) + 1.0
    dinv = (1.0 / np.sqrt(deg)).astype(np.float32)

    NPAD = cfg.NCP - cfg.NCR
    LO_REAL = cfg.LO_N - NPAD // 2       # real LO nodes per core
    score = src // cfg.NCR
    sloc = src % cfg.NCR
    dcore = dst // cfg.NCR
    dloc = dst % cfg.NCR
    lo_src = sloc < LO_REAL              # static class of each edge

    # local id -> half (including pads split between halves)
    own = np.arange(cfg.NCP)
    own_lo = (own < LO_REAL) | ((own >= cfg.NCR) &
                                (own < cfg.NCR + NPAD // 2))
    lo_ids = np.flatnonzero(own_lo)
    hi_ids = np.flatnonzero(~own_lo)
    assert len(lo_ids) == cfg.LO_N

    cores = []
    for c in range(cfg.NCORES):
        m = dcore == c
        cores.append(dict(mask=m, dl=dloc[m], lo=lo_src[m]))

    # pass 1: per-core, per-half window packing sorted by max(dLO,dHI)
    for cc in cores:
        dLO = np.bincount(cc["dl"][cc["lo"]], minlength=cfg.NCP)
        dHI = np.bincount(cc["dl"][~cc["lo"]], minlength=cfg.NCP)
        ordl = lo_ids[np.lexsort((-dHI[lo_ids], -dLO[lo_ids],
                                  -np.maximum(dLO, dHI)[lo_ids]))]
        ordh = hi_ids[np.lexsort((-dHI[hi_ids], -dLO[hi_ids],
                                  -np.maximum(dLO, dHI)[hi_ids]))]
        cc.update(dLO=dLO, dHI=dHI, order0=np.concatenate([ordl, ordh]))

    # shared per-bin round maxima (bin w = order0[w*128:(w+1)*128])
    RLO0 = np.zeros(cfg.NW, np.int64)
    RHI0 = np.zeros(cfg.NW, np.int64)
    for cc in cores:
        RLO0 = np.maximum(RLO0, cc["dLO"][cc["order0"]].reshape(cfg.NW, 128).max(1))
        RHI0 = np.maximum(RHI0, cc["dHI"][cc["order0"]].reshape(cfg.NW, 128).max(1))
    RLO0 = np.maximum(RLO0, 1)

    # half-pure call groups (LO-id windows never share a group with HI-id
    # windows), balanced by load within each half. In the HI phase the
    # LO-id groups' spans are gathered FIRST so all LO-id windows finish
    # early: that releases the layer-2 LO AllGather mid-scatter.
    def balance(bins, loads_):
        ng = (len(bins) + cfg.GRPW - 1) // cfg.GRPW
        gl = np.zeros(ng, np.int64)
        gs = [[] for _ in range(ng)]
        for b in sorted(bins, key=lambda b: -loads_[b]):
            gi = min((g for g in range(ng) if len(gs[g]) < cfg.GRPW),
                     key=lambda g: gl[g])
            gs[gi].append(int(b))
            gl[gi] += loads_[b]
        return gs

    loads = RLO0 + RHI0
    groups_lo = balance(range(cfg.NLO_W), loads)
    # the 5 lightest HI bins form their own FINAL group so the last
    # emission (and its softmax chunk) is as small as possible
    groups_hi = balance(range(cfg.NLO_W, cfg.NW - 5), loads)
    groups_hi.append(list(range(cfg.NW - 5, cfg.NW)))
    groups = groups_lo + groups_hi
    RLO, RHI = RLO0, RHI0

    # final per-core node order is just order0 (bin w = window w)
    for cc in cores:
        order = cc["order0"]
        qpos = np.empty(cfg.NCP, np.int64)
        qpos[order] = np.arange(cfg.NCP)
        cc.update(order=order, qpos=qpos)

    # global slot layout: LO phase (per group, LO spans), then HI phase
    calls = []
    win_lo = [[] for _ in range(cfg.NW)]
    win_hi = [[] for _ in range(cfg.NW)]
    off = 0
    NCALL_LO = 0
    for region, RR, win_c in ((0, RLO, win_lo), (1, RHI, win_hi)):
        for g in groups:
            span_rows = [(wdw, r) for wdw in g for r in range(int(RR[wdw]))]
            rows = len(span_rows)
            if not rows:
                continue
            ncall = (rows * 128 + cfg.CHUNK - 1) // cfg.CHUNK
            per = (rows + ncall - 1) // ncall
            r0 = 0
            while r0 < rows:
                n_rows = min(per, rows - r0)
                ci = len(calls)
                calls.append((off, n_rows * 128, region))
                for rr in range(n_rows):
                    wdw, _ = span_rows[r0 + rr]
                    win_c[wdw].append((ci, rr))
                off += n_rows * 128
                r0 += n_rows
        if region == 0:
            NCALL_LO = len(calls)
    SLOTS = off
    assert SLOTS % 16 == 0

    emit_order = [w for g in groups for w in g]
    meta = Meta(calls=calls, win_lo=win_lo, win_hi=win_hi,
                emit_order=emit_order, SLOTS=SLOTS, NCALL_LO=NCALL_LO)

    # pass 2: fill per-core slot index arrays.
    # region-local table row of source (c, local q0):
    #   LO: c*LO_N + qpos[q0]            (qpos < LO_N)
    #   HI: c*(NCP-LO_N) + qpos[q0]-LO_N
    qpos_all = np.concatenate([cc["qpos"] for cc in cores])
    qp_src = qpos_all[score * cfg.NCP + sloc]
    HI_N = cfg.NCP - cfg.LO_N
    grow = np.where(lo_src, score * cfg.LO_N + qp_src,
                    score * HI_N + qp_src - cfg.LO_N)

    zrowLO = int(cores[0]["qpos"][cfg.NCR])            # core0 LO pad
    zrowHI = int(cores[0]["qpos"][cfg.NCR + NPAD // 2]) - cfg.LO_N
    assert 0 <= zrowLO < cfg.LO_N and 0 <= zrowHI < HI_N

    # global slot of each (window, class round)
    rsL = np.zeros((cfg.NW, int(RLO.max())), np.int64)
    rsH = np.zeros((cfg.NW, max(int(RHI.max()), 1)), np.int64)
    for wdw in range(cfg.NW):
        for r, (ci, rr) in enumerate(win_lo[wdw]):
            rsL[wdw, r] = calls[ci][0] + rr * 128
        for r, (ci, rr) in enumerate(win_hi[wdw]):
            rsH[wdw, r] = calls[ci][0] + rr * 128

    for c in range(cfg.NCORES):
        cc = cores[c]
        m = cc["mask"]
        eg = grow[m]
        elo = cc["lo"]
        edl = cc["dl"]
        ew = cc["qpos"][edl] // 128
        ep = cc["qpos"][edl] % 128
        # rank of edge within its (dloc, class) group
        key = edl * 2 + (~elo).astype(np.int64)
        o = np.argsort(key, kind="stable")
        ks = key[o]
        first = np.r_[0, np.flatnonzero(ks[1:] != ks[:-1]) + 1]
        starts = np.zeros(len(ks), np.int64)
        starts[first] = first
        starts = np.maximum.accumulate(starts)
        rank = np.empty(len(ks), np.int64)
        rank[o] = np.arange(len(ks)) - starts

        idxv = np.full(SLOTS, -1, np.int64)
        for coff, n, region in calls:
            idxv[coff:coff + n] = zrowLO if region == 0 else zrowHI
        posL = rsL[ew, np.minimum(rank, rsL.shape[1] - 1)] + ep
        posH = rsH[ew, np.minimum(rank, rsH.shape[1] - 1)] + ep
        pos = np.where(elo, posL, posH)
        idxv[pos] = eg
        assert idxv.min() >= 0
        assert idxv[np.concatenate([np.arange(co, co + n)
                    for co, n, rg in calls if rg == 0])].max() < cfg.LOTOT
        gi = idxv.reshape(-1, 16).T.astype(np.int16)         # [16, SLOTS/16]
        cc["gidx"] = np.tile(gi, (8, 1))                     # [128, SLOTS/16]
    return dinv, cores, meta


def prep_inputs(cfg: Cfg, inputs: dict, dinv, cores, meta: Meta):
    """Build per-core in_maps (numpy) for the device kernel."""
    x = np.asarray(inputs["x"], np.float32)
    lin_W = np.asarray(inputs["lin_W"], np.float32)
    lin_b = np.asarray(inputs["lin_b"], np.float32)
    W1 = np.asarray(inputs["W1"], np.float32)
    b1 = np.asarray(inputs["b1"], np.float32)
    W2 = np.asarray(inputs["W2"], np.float32)
    b2 = np.asarray(inputs["b2"], np.float32)

    linWp = np.zeros((cfg.FP, cfg.H1), BF16)
    linWp[:cfg.FIN] = lin_W.astype(BF16)
    nf1 = cfg.H1 // cfg.F1C
    linbp = lin_b.reshape(nf1, cfg.F1C).T.astype(np.float32).copy()  # [F1C, nf1]
    W1p = np.zeros((cfg.H1, cfg.H2P), BF16)
    W1p[:, :cfg.H2] = W1.astype(BF16)
    W2p = np.zeros((cfg.H2P, cfg.C), BF16)
    W2p[:cfg.H2] = W2.astype(BF16)
    b1rep = np.zeros((128, cfg.H2P), np.float32)
    b1rep[:, :cfg.H2] = b1
    b2rep = np.tile(b2.reshape(1, cfg.C), (128, 1)).astype(np.float32)
    ident = np.eye(128, dtype=BF16)

    xT = np.zeros((cfg.FP, cfg.N), np.float32)
    xT[:cfg.FIN] = x.T

    in_maps = []
    for c in range(cfg.NCORES):
        cc = cores[c]
        order = cc["order"]
        real = order < cfg.NCR
        gcols = np.where(real, cfg.NCR * c + np.minimum(order, cfg.NCR - 1), 0)
        xTc = xT[:, gcols] * real[None, :]
        dv = dinv[gcols] * real
        dinvT = np.tile(dv.astype(BF16).reshape(1, -1), (128, 1))
        dinvN = dv.reshape(cfg.NW, 128).T.astype(np.float32).copy()
        in_maps.append({
            "xT": xTc.astype(BF16),
            "linW": linWp, "linb": linbp,
            "W1": W1p, "W2": W2p,
            "b1rep": b1rep, "b2rep": b2rep,
            "dinvT": dinvT, "dinvN": dinvN,
            "ident": ident, "gidx": cc["gidx"],
        })
    return in_maps


def assemble_output(cfg: Cfg, cores, outs):
    """outs: per-core [128, NW*C] -> full [N, C] float32."""
    res = np.empty((cfg.N, cfg.C), np.float32)
    for c in range(cfg.NCORES):
        o = np.asarray(outs[c]).reshape(128, cfg.NW, cfg.C)
        o = o.transpose(1, 0, 2).reshape(cfg.NCP, cfg.C)  # device node order
        order = cores[c]["order"]
        real = order < cfg.NCR
        res[c * cfg.NCR + order[real]] = o[real]
    return res


# ---------------------------------------------------------------- device kernel


def build_nc(cfg: Cfg, meta: Meta):
    import concourse.bacc as bacc
    import concourse.mybir as mybir
    import concourse.tile as tile

    dt = mybir.dt
    f32, bf16, i16 = dt.float32, dt.bfloat16, dt.int16
    fp8 = dt.float8e4
    AF = mybir.ActivationFunctionType
    OP = mybir.AluOpType

    nc = bacc.Bacc("TRN2", target_bir_lowering=False, debug=False,
                   enable_asserts=False, num_devices=cfg.NCORES,
                   num_swdge_queues=2,
                   dynamic_dma_scratch_size=cfg.SCRATCH)

    NCP, NW, NTOT, C = cfg.NCP, cfg.NW, cfg.NTOT, cfg.C
    LO_N, LOTOT, HITOT = cfg.LO_N, cfg.LOTOT, cfg.HITOT
    NLO_W = cfg.NLO_W
    F1C, H2P, H2 = cfg.F1C, cfg.H2P, cfg.H2
    NK = cfg.FP // 128          # contraction tiles for GEMM1
    NF1 = cfg.H1 // F1C         # feature chunks of h1

    xT_d = nc.dram_tensor("xT", [cfg.FP, NCP], bf16, kind="ExternalInput")
    linW_d = nc.dram_tensor("linW", [cfg.FP, cfg.H1], bf16, kind="ExternalInput")
    linb_d = nc.dram_tensor("linb", [F1C, NF1], f32, kind="ExternalInput")
    W1_d = nc.dram_tensor("W1", [cfg.H1, H2P], bf16, kind="ExternalInput")
    W2_d = nc.dram_tensor("W2", [H2P, C], bf16, kind="ExternalInput")
    b1r_d = nc.dram_tensor("b1rep", [128, H2P], f32, kind="ExternalInput")
    b2r_d = nc.dram_tensor("b2rep", [128, C], f32, kind="ExternalInput")
    dvT_d = nc.dram_tensor("dinvT", [128, NCP], bf16, kind="ExternalInput")
    dvN_d = nc.dram_tensor("dinvN", [128, NW], f32, kind="ExternalInput")
    id_d = nc.dram_tensor("ident", [128, 128], bf16, kind="ExternalInput")
    gi_d = nc.dram_tensor("gidx", [128, meta.SLOTS // 16], i16, kind="ExternalInput")
    out_d = nc.dram_tensor("out", [128, NW * C], f32, kind="ExternalOutput")

    GW = [(i, min(512, NCP - i)) for i in range(0, NCP, 512)]
    CROWS = cfg.CHUNK // 128
    outv = out_d[:].rearrange("p (w c) -> p w c", c=C)

    with tile.TileContext(nc) as tc, ExitStack() as top:
        const = top.enter_context(tc.tile_pool(name="const", bufs=1))
        dram = top.enter_context(tc.tile_pool(name="dram", bufs=1, space="DRAM"))

        ident = const.tile([128, 128], bf16)
        nc.sync.dma_start(ident[:], id_d[:])
        dinvT = const.tile([128, NCP], bf16)
        nc.sync.dma_start(dinvT[:], dvT_d[:])
        dinvN = const.tile([128, NW], f32)
        nc.sync.dma_start(dinvN[:], dvN_d[:])
        b1rep = const.tile([128, H2P], f32)
        nc.sync.dma_start(b1rep[:], b1r_d[:])
        b2rep = const.tile([128, C], f32)
        nc.sync.dma_start(b2rep[:], b2r_d[:])
        linb = const.tile([F1C, NF1], f32)
        nc.sync.dma_start(linb[:], linb_d[:])
        gidx = const.tile([128, meta.SLOTS // 16], i16)
        nc.sync.dma_start(gidx[:], gi_d[:])
        W1t = []
        for f in range(NF1):
            t = const.tile([F1C, H2P], bf16, name=f"W1t{f}")
            nc.sync.dma_start(t[:], W1_d[f * F1C:(f + 1) * F1C, :])
            W1t.append(t)
        W2t = const.tile([H2P, C], bf16)
        nc.sync.dma_start(W2t[:], W2_d[:])

        g1T = const.tile([128, NCP], bf16, tag="bigA")
        g1nat = const.tile([128, NW, 128], bf16, tag="bigB")
        g1nat8 = const.tile([128, NW, H2], fp8)
        h2nat = const.tile([128, NW, H2P], bf16)
        h2T = const.tile([128, NW, 128], bf16, tag="bigA")  # [f, w, p]
        g2Tf = const.tile([128, NCP], bf16)
        g2nat = const.tile([128, NW, 128], bf16, tag="bigB")
        g2nat8 = const.tile([128, NW, C], fp8)
        logit = const.tile([128, NW, C], f32)
        outsb = const.tile([128, NW, C], f32)
        h1part = const.tile([128, NW, H2], f32)
        l2part = const.tile([128, NW, C], f32)

        g1loc8a = dram.tile([LO_N, H2], fp8)
        g1loc8b = dram.tile([NCP - LO_N, H2], fp8)
        g2loc8a = dram.tile([LO_N, C], fp8)
        g2loc8b = dram.tile([NCP - LO_N, C], fp8)
        full1p8a = dram.tile([LOTOT, H2], fp8, addr_space="Shared")
        full1p8b = dram.tile([HITOT, H2], fp8, addr_space="Shared")
        full18 = dram.tile([NTOT, 256], fp8)
        full2p8a = dram.tile([LOTOT, C], fp8, addr_space="Shared")
        full2p8b = dram.tile([HITOT, C], fp8, addr_space="Shared")
        full28 = dram.tile([NTOT, 256], fp8)

        # garbage-free upper feature columns for the h2 transpose / GEMM3
        nc.gpsimd.memset(h2nat[:], 0.0)
        nc.gpsimd.memset(g2Tf[:], 0.0)

        # ---- phase A: fused GEMM1 (relu(x@linW+b)) + GEMM2 (g1 = dinv*(h1@W1)),
        # with the g1 transpose/fp8-convert/table-write interleaved per group
        with ExitStack() as ph:
            xp = ph.enter_context(tc.tile_pool(name="xp", bufs=3))
            hp = ph.enter_context(tc.tile_pool(name="hp", bufs=2))
            psA = ph.enter_context(tc.tile_pool(name="psA", bufs=2, space="PSUM"))
            psB = ph.enter_context(tc.tile_pool(name="psB", bufs=2, space="PSUM"))
            lw = ph.enter_context(tc.tile_pool(name="lw", bufs=1))
            lwt = []
            for k in range(NK):
                t = lw.tile([128, cfg.H1], bf16, name=f"lwt{k}")
                nc.sync.dma_start(t[:], linW_d[k * 128:(k + 1) * 128, :])
                lwt.append(t)
            xTv = xT_d[:].rearrange("(k p) c -> p k c", p=128)
            for (c0, cw) in GW:
                xg = xp.tile([128, NK, 512], bf16, tag="xg")
                nc.sync.dma_start(xg[:, :, :cw], xTv[:, :, c0:c0 + cw])
                h1g = hp.tile([F1C, NF1, 512], bf16, tag="h1g")
                for f in range(NF1):
                    acc = psA.tile([F1C, 512], f32, tag="accA")
                    for k in range(NK):
                        nc.tensor.matmul(
                            acc[:, :cw],
                            lwt[k][:, f * F1C:(f + 1) * F1C],
                            xg[:, k, :cw],
                            start=(k == 0), stop=(k == NK - 1))
                    nc.scalar.activation(h1g[:, f, :cw], acc[:, :cw],
                                         AF.Relu, bias=linb[:, f:f + 1])
                accB = psB.tile([H2P, 512], f32, tag="accB")
                for f in range(NF1):
                    nc.tensor.matmul(accB[:, :cw], W1t[f][:],
                                     h1g[:, f, :cw],
                                     start=(f == 0), stop=(f == NF1 - 1))
                nc.vector.tensor_mul(g1T[:, c0:c0 + cw], accB[:, :cw],
                                     dinvT[:, c0:c0 + cw])
                if c0 == LO_N + 1536:
                    # LO half of the table completed a few groups ago; its
                    # tail ops' waits are satisfied by now, so emitting them
                    # here does not stall the x prefetch queue
                    nc.sync.dma_start_transpose(g1nat[:, :NLO_W, :],
                                                g1T[:, :LO_N])
                    nc.vector.tensor_copy(g1nat8[:, :NLO_W, :],
                                          g1nat[:, :NLO_W, :H2])
                    nc.sync.dma_start(
                        g1loc8a[:].rearrange("(w p) f -> p w f", p=128),
                        g1nat8[:, :NLO_W, :])
                    nc.gpsimd.collective_compute(
                        "AllGather", OP.bypass,
                        replica_groups=[list(range(cfg.NCORES))],
                        ins=[g1loc8a[:]], outs=[full1p8a[:]])

        # b-chain on the Activation HWDGE queue: the SP queue's scheduler
        # barriers would otherwise serialize it behind AllGather_a
        nc.scalar.dma_start_transpose(g1nat[:, NLO_W:, :], g1T[:, LO_N:])
        nc.vector.tensor_copy(g1nat8[:, NLO_W:, :], g1nat[:, NLO_W:, :H2])
        nc.scalar.dma_start(
            g1loc8b[:].rearrange("(w p) f -> p w f", p=128),
            g1nat8[:, NLO_W:, :])
        nc.gpsimd.collective_compute(
            "AllGather", OP.bypass,
            replica_groups=[list(range(cfg.NCORES))],
            ins=[g1loc8b[:]], outs=[full1p8b[:]])
        # expand packed rows into 256 B-stride gather rows; LO expand runs
        # during AllGather_b, so LO-class gathers also overlap AllGather_b
        nc.sync.dma_start(full18[:LOTOT, :H2], full1p8a[:])
        nc.sync.dma_start(full18[LOTOT:, :H2], full1p8b[:])

        # ---- log_softmax over C for a window range (bulk: 3 act-table
        # loads per chunk, not per window)
        def softmax_chunk(sp, w0, w1):
            nwc = w1 - w0
            et = sp.tile([128, NW, C], f32, tag="et")
            nc.scalar.activation(et[:, :nwc, :], logit[:, w0:w1, :], AF.Exp)
            ssum = sp.tile([128, NW], f32, tag="ssum")
            nc.vector.tensor_reduce(ssum[:, :nwc], et[:, :nwc, :],
                                    mybir.AxisListType.X, OP.add)
            negl = sp.tile([128, NW], f32, tag="negl")
            nc.scalar.activation(negl[:, :nwc], ssum[:, :nwc], AF.Ln)
            nc.vector.tensor_scalar_mul(negl[:, :nwc], negl[:, :nwc], -1.0)
            for w in range(w0, w1):
                nc.scalar.activation(outsb[:, w, :], logit[:, w, :],
                                     AF.Identity, bias=negl[:, w - w0:w - w0 + 1])
            nc.sync.dma_start(outv[:, w0:w1, :], outsb[:, w0:w1, :])

        # layer-2 projection chain for a window range: h2 -> h2T -> GEMM3 ->
        # g2 (T) -> g2 natural -> fp8 -> local table slice
        ps3 = top.enter_context(tc.tile_pool(name="ps3", bufs=2, space="PSUM"))

        def g2_chain(w0, w1):
            nc.sync.dma_start_transpose(
                h2T[:, w0:w1, :],
                h2nat[:, w0:w1, :].rearrange("p w f -> p (w f)"))
            h2Tf = h2T[:].rearrange("f w p -> f (w p)")
            for c0 in range(w0 * 128, w1 * 128, 512):
                cw = min(512, w1 * 128 - c0)
                acc = ps3.tile([C, 512], f32, tag="acc3")
                nc.tensor.matmul(acc[:, :cw], W2t[:], h2Tf[:, c0:c0 + cw],
                                 start=True, stop=True)
                nc.vector.tensor_mul(g2Tf[:C, c0:c0 + cw], acc[:, :cw],
                                     dinvT[:C, c0:c0 + cw])
            nc.sync.dma_start_transpose(g2nat[:, w0:w1, :],
                                        g2Tf[:, w0 * 128:w1 * 128])
            nc.vector.tensor_copy(g2nat8[:, w0:w1, :],
                                  g2nat[:, w0:w1, :C])
            g2dst = g2loc8a if w1 <= NLO_W else g2loc8b
            woff = 0 if w1 <= NLO_W else NLO_W
            nc.sync.dma_start(
                g2dst[(w0 - woff) * 128:(w1 - woff) * 128, :]
                .rearrange("(w p) c -> p w c", p=128),
                g2nat8[:, w0:w1, :])

        def emit_ag2a():
            nc.gpsimd.collective_compute(
                "AllGather", OP.bypass,
                replica_groups=[list(range(cfg.NCORES))],
                ins=[g2loc8a[:]], outs=[full2p8a[:]])

        # ---- scatter layers (two phases: LO-class gathers into per-window
        # partials, then HI-class gathers + combine)
        def scatter(full, gnat, layer, fw, part):
            with ExitStack() as ph:
                gp = ph.enter_context(tc.tile_pool(name=f"gb{layer}", bufs=5))
                pp = ph.enter_context(tc.tile_pool(name=f"psW{layer}", bufs=4,
                                                   space="PSUM"))
                ep = ph.enter_context(tc.tile_pool(name=f"ep{layer}", bufs=6))
                sp = ph.enter_context(tc.tile_pool(name=f"sm{layer}", bufs=1))
                wcut = NW - 5               # last softmax chunk (light group)
                gtiles = {}
                lo_done = hi_done = cnt_loid = 0
                cnt_a = cnt_b = 0
                defer_ag2a = -1
                for ci, (coff, n, region) in enumerate(meta.calls):
                    t = gp.tile([128, CROWS, 256], fp8, tag="gb")
                    src = full[:LOTOT, :] if region == 0 else full[LOTOT:, :]
                    nc.gpsimd.dma_gather(
                        t[:, :n // 128, :], src,
                        gidx[:, coff // 16:(coff + n) // 16],
                        num_idxs=n, num_idxs_reg=n, elem_size=256,
                        queue_num=ci % 2)
                    gtiles[ci] = t
                    if defer_ag2a > 0:
                        defer_ag2a -= 1
                        if defer_ag2a == 0:
                            emit_ag2a()
                    if region == 0:
                        # accumulate self + LO rounds into the partial
                        while lo_done < NW and all(
                                c <= ci for c, _ in
                                meta.win_lo[meta.emit_order[lo_done]]):
                            w = meta.emit_order[lo_done]
                            chunks = meta.win_lo[w]
                            acc = pp.tile([128, fw], f32, tag="pw")
                            nc.tensor.matmul(acc[:], ident[:], gnat[:, w, :fw],
                                             start=True, stop=not chunks)
                            for k, (cidx, row) in enumerate(chunks):
                                nc.tensor.matmul(
                                    acc[:], ident[:], gtiles[cidx][:, row, :fw],
                                    start=False, stop=(k == len(chunks) - 1))
                            nc.vector.tensor_copy(part[:, w, :], acc[:])
                            lo_done += 1
                        continue
                    while hi_done < NW and all(
                            c <= ci for c, _ in
                            meta.win_hi[meta.emit_order[hi_done]]):
                        w = meta.emit_order[hi_done]
                        chunks = meta.win_hi[w]
                        if chunks:
                            acc = pp.tile([128, fw], f32, tag="pw")
                            for k, (cidx, row) in enumerate(chunks):
                                nc.tensor.matmul(
                                    acc[:], ident[:], gtiles[cidx][:, row, :fw],
                                    start=(k == 0), stop=(k == len(chunks) - 1))
                            s = ep.tile([128, fw], f32, tag="s")
                            nc.vector.scalar_tensor_tensor(
                                s[:], acc[:], 0.0, part[:, w, :],
                                OP.add, OP.add)
                            sv = s[:]
                        else:
                            sv = part[:, w, :]
                        if layer == 1:
                            t2 = ep.tile([128, H2], f32, tag="t2")
                            nc.vector.scalar_tensor_tensor(
                                t2[:], sv, dinvN[:, w:w + 1],
                                b1rep[:, :H2], OP.mult, OP.add)
                            nc.scalar.activation(h2nat[:, w, :H2], t2[:],
                                                 AF.Relu)
                            if w < NLO_W:
                                cnt_loid += 1
                                if cnt_loid == NLO_W:
                                    # all LO-id windows done: project their
                                    # g2 slice; the AllGather is emitted a
                                    # few calls later so its input wait does
                                    # not stall the Pool gather queue
                                    g2_chain(0, NLO_W)
                                    defer_ag2a = 6
                        else:
                            nc.vector.scalar_tensor_tensor(
                                logit[:, w, :], sv, dinvN[:, w:w + 1],
                                b2rep[:], OP.mult, OP.add)
                            if w < NLO_W:
                                cnt_a += 1
                                if cnt_a == NLO_W:
                                    softmax_chunk(sp, 0, NLO_W)
                            elif w < wcut:
                                cnt_b += 1
                                if cnt_b == wcut - NLO_W:
                                    softmax_chunk(sp, NLO_W, wcut)
                        hi_done += 1
                assert lo_done == NW and hi_done == NW
                if layer == 1 and defer_ag2a > 0:
                    emit_ag2a()
                if layer == 2:
                    softmax_chunk(sp, wcut, NW)

        scatter(full18, g1nat, layer=1, fw=H2, part=h1part)

        # remaining HI-id window projection + its AllGather; the LO expand
        # (and then scatter-2's LO gathers) overlap AllGather_b of layer 2
        g2_chain(NLO_W, NW)
        nc.gpsimd.collective_compute(
            "AllGather", OP.bypass,
            replica_groups=[list(range(cfg.NCORES))],
            ins=[g2loc8b[:]], outs=[full2p8b[:]])
        nc.sync.dma_start(full28[:LOTOT, :C], full2p8a[:])
        nc.sync.dma_start(full28[LOTOT:, :C], full2p8b[:])

        scatter(full28, g2nat, layer=2, fw=C, part=l2part)

    nc.compile()
    return nc


# ---------------------------------------------------------------- entry

_CACHE = {}


def _get_nc(cfg: Cfg, meta: Meta):
    key = (cfg.N, cfg.E, meta.SLOTS, tuple(tuple(c) for c in meta.calls))
    if key not in _CACHE:
        _CACHE[key] = build_nc(cfg, meta)
    return _CACHE[key]


def run(cfg: Cfg, inputs: dict, trace: bool = False):
    from concourse.bass_utils import run_bass_kernel_spmd
    dinv, cores, meta = prep_graph(cfg, np.asarray(inputs["edge_index"]))
    in_maps = prep_inputs(cfg, inputs, dinv, cores, meta)
    nc = _get_nc(cfg, meta)
    try:
        res = run_bass_kernel_spmd(nc, in_maps,
                                   core_ids=list(range(cfg.NCORES)),
                                   trace=trace)
    except ModuleNotFoundError:
        res = run_bass_kernel_spmd(nc, in_maps,
                                   core_ids=list(range(cfg.NCORES)),
                                   trace=False)
    out = assemble_output(cfg, cores, [r["out"] for r in res.results])
    return out, res


def kernel(**inputs) -> np.ndarray:
    out, _ = run(FULL, inputs)
    return out


def bench_chain(cfg: Cfg, inputs: dict, iters: int = 8):
    """Time device execution by chaining `iters` NEFF executions in one jit
    (output of run k feeds the donated output buffer of run k+1, serializing
    them); returns (per_exec_seconds, outputs_of_last_run)."""
    import time as _time

    import jax
    import numpy as _np
    from jax.experimental.shard_map import shard_map
    from jax.sharding import Mesh, PartitionSpec

    import concourse.mybir as mybir
    from concourse import bass2jax

    dinv, cores, meta = prep_graph(cfg, np.asarray(inputs["edge_index"]))
    in_maps = prep_inputs(cfg, inputs, dinv, cores, meta)
    nc = _get_nc(cfg, meta)
    bass2jax.install_neuronx_cc_hook()

    pname = nc.partition_id_tensor.name if nc.partition_id_tensor else None
    in_names, out_names, out_avals, zero_outs = [], [], [], []
    for alloc in nc.m.functions[0].allocations:
        if not isinstance(alloc, mybir.MemoryLocationSet):
            continue
        name = alloc.memorylocations[0].name
        if alloc.kind == "ExternalInput":
            if name != pname:
                in_names.append(name)
        elif alloc.kind == "ExternalOutput":
            out_names.append(name)
            shape = tuple(alloc.tensor_shape)
            dtype = mybir.dt.np(alloc.dtype)
            out_avals.append(jax.core.ShapedArray(shape, dtype))
            zero_outs.append(_np.zeros(shape, dtype))
    n_params = len(in_names)
    all_names = in_names + out_names + ([pname] if pname else [])

    def _body_n(n_execs, *args):
        operands = list(args[:n_params])
        outs = list(args[n_params:])
        pid = [bass2jax.partition_id_tensor()] if pname else []
        for _ in range(n_execs):
            outs = list(bass2jax._bass_exec_p.bind(
                *operands, *outs, *pid,
                out_avals=tuple(out_avals),
                in_names=tuple(all_names),
                out_names=tuple(out_names),
                lowering_input_output_aliases=(),
                sim_require_finite=True, sim_require_nnan=True, nc=nc))
        return tuple(outs)

    devices = jax.devices()[:cfg.NCORES]
    mesh = Mesh(_np.asarray(devices), ("core",))
    spec = (PartitionSpec("core"),)
    concat_in = [_np.concatenate([_np.asarray(in_maps[c][n])
                                  for c in range(cfg.NCORES)], axis=0)
                 for n in in_names]
    concat_zeros = [_np.zeros((cfg.NCORES * z.shape[0], *z.shape[1:]), z.dtype)
                    for z in zero_outs]
    nin = n_params + len(zero_outs)
    fn = jax.jit(
        shard_map(lambda *a: _body_n(1, *a), mesh=mesh,
                  in_specs=spec * nin, out_specs=spec * len(out_names),
                  check_rep=False),
        donate_argnums=tuple(range(n_params, nin)), keep_unused=True)
    din = [jax.device_put(x) for x in concat_in]
    outs = fn(*din, *concat_zeros)  # compile+warm
    jax.block_until_ready(outs)
    zzs = []
    for _ in range(iters):
        zzs.append([jax.device_put(
            _np.zeros((cfg.NCORES * z.shape[0], *z.shape[1:]), z.dtype))
            for z in zero_outs])
    jax.block_until_ready(zzs)
    t0 = _time.perf_counter()
    all_outs = [fn(*din, *zz) for zz in zzs]
    jax.block_until_ready(all_outs)
    t_pipe = _time.perf_counter() - t0
    times = [t_pipe / iters]
    per_exec = times[0]
    last = all_outs[-1]
    outs_np = [_np.asarray(last[i]).reshape(cfg.NCORES, *out_avals[i].shape)
               for i in range(len(out_names))]
    out = assemble_output(cfg, cores,
                          [outs_np[out_names.index("out")][c]
                           for c in range(cfg.NCORES)])
    return per_exec, times, out

